# revision 1
# baseline (speedup 1.0000x reference)
"""GQA (RoPE + causal softmax) Trainium2 Bass kernel, 8-core SPMD.

Sharding: DP over batch (2) x TP over KV groups (4 quarters of heads).
Core c handles batch c//4 and head quarter c%4 (8 q-heads, 2 kv-heads).
Each core computes a partial o_proj ([S, D]); host sums 4 partials per batch.

All matmuls run in float32r (TF32-like, 1 cyc/row at N>=256).
Everything on-chip is kept in "transposed" layout (feature dim on
partitions), which makes x^T the only host-side layout prep needed.
"""

import numpy as np

import concourse.bass as bass
import concourse.mybir as mybir
import concourse.tile as tile
from concourse import bacc, bass_utils

B, S, D = 2, 2048, 2048
H, KV, HD = 32, 8, 64
REP = H // KV
SCALE = 1.0 / 8.0  # 1/sqrt(HD)

F32 = mybir.dt.float32
F32R = mybir.dt.float32r
EXP = mybir.ActivationFunctionType.Exp

NCHUNK = S // 512        # 4 sq chunks of 512
NKT = D // 128           # 16 k-tiles over D
NST = S // 128           # 16 sk/s tiles

# local head j (0..7) -> denom row
def _pairrow(j):
    return 2 * (j % 4) + (j // 4)


def _build_program():
    nc = bacc.Bacc()

    xT = nc.dram_tensor("xT", [D, S], F32R, kind="ExternalInput").ap()
    wq = nc.dram_tensor("wq", [D, 8 * HD], F32R, kind="ExternalInput").ap()
    wk = nc.dram_tensor("wk", [D, 2 * HD], F32R, kind="ExternalInput").ap()
    wv = nc.dram_tensor("wv", [D, 2 * HD], F32R, kind="ExternalInput").ap()
    wo = nc.dram_tensor("wo", [8 * HD, D], F32R, kind="ExternalInput").ap()
    cosT2 = nc.dram_tensor("cosT2", [128, S], F32, kind="ExternalInput").ap()
    sinT2m = nc.dram_tensor("sinT2m", [128, S], F32, kind="ExternalInput").ap()
    tri = nc.dram_tensor("tri", [128, 128], F32, kind="ExternalInput").ap()
    ident = nc.dram_tensor("ident", [128, 64], F32R, kind="ExternalInput").ap()
    selA = nc.dram_tensor("selA", [128, 512], F32R, kind="ExternalInput").ap()
    selB = nc.dram_tensor("selB", [128, 512], F32R, kind="ExternalInput").ap()
    onescol = nc.dram_tensor("onescol", [128, 1], F32R, kind="ExternalInput").ap()
    zblk = nc.dram_tensor("zblk", [128, 128], F32R, kind="ExternalInput").ap()
    opart = nc.dram_tensor("opart", [S, D], F32, kind="ExternalOutput").ap()

    with tile.TileContext(nc) as tc:
        with (
            tc.tile_pool(name="persist", bufs=1) as pp,
            tc.tile_pool(name="consts", bufs=1) as cp,
        ):
            # persistent SBUF: q^T/k^T, attention outputs, small constants
            qT = [pp.tile([128, S], F32R, tag=f"qT{t}", name=f"qT{t}") for t in range(4)]
            kT = pp.tile([128, S], F32R, tag="kT")
            outT = [pp.tile([128, S], F32R, tag=f"outT{t}", name=f"outT{t}") for t in range(4)]
            denomA = pp.tile([128, S], F32, tag="denomA")
            denomB = pp.tile([128, S], F32, tag="denomB")
            trib = cp.tile([128, 128], F32, tag="trib")
            identb = cp.tile([128, 64], F32R, tag="identb")
            selAb = cp.tile([128, 512], F32R, tag="selAb")
            selBb = cp.tile([128, 512], F32R, tag="selBb")
            onesb = cp.tile([128, 1], F32R, tag="onesb")
            zblkb = cp.tile([128, 128], F32R, tag="zblkb")
            nc.sync.dma_start(trib[:], tri[:])
            nc.sync.dma_start(identb[:], ident[:])
            nc.sync.dma_start(selAb[:], selA[:])
            nc.sync.dma_start(selBb[:], selB[:])
            nc.sync.dma_start(onesb[:], onescol[:])
            nc.sync.dma_start(zblkb[:], zblk[:])
            nc.gpsimd.memset(denomA[:], 1.0)
            nc.gpsimd.memset(denomB[:], 1.0)

            vo = [[None] * NST, [None] * NST]
            with tc.tile_pool(name="vop", bufs=1) as vp:  # spans phases A..D
                with (
                    tc.tile_pool(name="ropec", bufs=1) as rcc,
                    tc.tile_pool(name="vtbuf", bufs=1) as vtb,
                ):
                    cosb = rcc.tile([128, S], F32, tag="cosb")
                    sinb = rcc.tile([128, S], F32, tag="sinb")
                    nc.sync.dma_start(cosb[:], cosT2[:])
                    nc.sync.dma_start(sinb[:], sinT2m[:])
                    vT = vtb.tile([128, S], F32R, tag="vT")

                    # ---------- Phase A: qkv^T = W^T @ x^T ----------
                    with (
                        tc.tile_pool(name="wts", bufs=1) as wp,
                        tc.tile_pool(name="xin", bufs=4) as xp,
                        tc.tile_pool(name="qkvps", bufs=6, space="PSUM") as pqkv,
                    ):
                        wqk = [wp.tile([128, 8 * HD], F32R, tag=f"wq{k}", name=f"wqk{k}") for k in range(NKT)]
                        wkk = [wp.tile([128, 2 * HD], F32R, tag=f"wk{k}", name=f"wkk{k}") for k in range(NKT)]
                        wvk = [wp.tile([128, 2 * HD], F32R, tag=f"wv{k}", name=f"wvk{k}") for k in range(NKT)]
                        for k in range(NKT):
                            r = slice(k * 128, (k + 1) * 128)
                            nc.sync.dma_start(wqk[k][:], wq[r, :])
                            nc.sync.dma_start(wkk[k][:], wk[r, :])
                            nc.sync.dma_start(wvk[k][:], wv[r, :])
                        for n in range(NCHUNK):
                            ncol = slice(n * 512, (n + 1) * 512)
                            accs = [pqkv.tile([128, 512], F32, tag="qkvacc", name=f"acc{n}_{m}") for m in range(6)]
                            for k in range(NKT):
                                xk = xp.tile([128, 512], F32R, tag="xk")
                                nc.sync.dma_start(xk[:], xT[k * 128:(k + 1) * 128, ncol])
                                st = k == 0
                                sp = k == NKT - 1
                                for t in range(4):
                                    nc.tensor.matmul(
                                        accs[t][:], wqk[k][:, t * 128:(t + 1) * 128],
                                        xk[:], start=st, stop=sp)
                                nc.tensor.matmul(accs[4][:], wkk[k][:], xk[:], start=st, stop=sp)
                                nc.tensor.matmul(accs[5][:], wvk[k][:], xk[:], start=st, stop=sp)
                            for t in range(4):
                                nc.vector.tensor_copy(qT[t][:, ncol], accs[t][:])
                            nc.vector.tensor_copy(kT[:, ncol], accs[4][:])
                            nc.vector.tensor_copy(vT[:, ncol], accs[5][:])

                    # ---------- Phase B: RoPE on q^T and k^T ----------
                    with tc.tile_pool(name="rope", bufs=2) as rp:
                        for tl in [*qT, kT]:
                            rot = rp.tile([128, S], F32, tag="rot")
                            tmp = rp.tile([128, S], F32, tag="tmp")
                            # rotate-half as partition-shifted copies (sign folded in sinb)
                            nc.gpsimd.tensor_copy(rot[0:32, :], tl[32:64, :])
                            nc.gpsimd.tensor_copy(rot[32:64, :], tl[0:32, :])
                            nc.gpsimd.tensor_copy(rot[64:96, :], tl[96:128, :])
                            nc.gpsimd.tensor_copy(rot[96:128, :], tl[64:96, :])
                            nc.vector.tensor_mul(tmp[:], tl[:], cosb[:])
                            nc.vector.tensor_mul(rot[:], rot[:], sinb[:])
                            nc.vector.tensor_add(tl[:], tmp[:], rot[:])

                    # ---------- Phase C: v natural tiles [128, 65] ----------
                    with tc.tile_pool(name="vtp", bufs=2, space="PSUM") as vtp:
                        for g in range(2):
                            for i in range(NST):
                                vps = vtp.tile([128, 64], F32R, tag="vps")
                                nc.tensor.transpose(
                                    vps[:], vT[g * 64:(g + 1) * 64, i * 128:(i + 1) * 128],
                                    identb[g * 64:(g + 1) * 64, :])
                                vt = vp.tile([128, 65], F32R, tag=f"vo{g}_{i}", name=f"vo{g}_{i}")
                                nc.vector.tensor_copy(vt[:, 0:64], vps[:])
                                nc.vector.tensor_copy(vt[:, 64:65], onesb[:])
                                vo[g][i] = vt

                # ---------- Phase D: attention ----------
                with (
                    tc.tile_pool(name="esb", bufs=10) as ep,
                    tc.tile_pool(name="sps", bufs=4, space="PSUM") as sp_,
                    tc.tile_pool(name="avp", bufs=3, space="PSUM") as ap_,
                ):
                    for t in range(4):
                        for j in range(NCHUNK):
                            jcol = slice(j * 512, (j + 1) * 512)
                            avs = []
                            for sub in range(2):
                                avs.append(ap_.tile([65, 512], F32, tag="avacc", name=f"av{t}_{j}_{sub}"))
                            for i in range(4 * j + 4):
                                c0 = max(0, 128 * (i - 4 * j))
                                ec0 = c0 if 512 - c0 >= 256 else 256
                                av0 = c0 if c0 < 384 else 256
                                for sub in range(2):
                                    pb = slice(64 * sub, 64 * sub + 64)
                                    g = sub
                                    ss = sp_.tile([128, 512], F32, tag="scps")
                                    nc.tensor.matmul(
                                        ss[:, ec0:512],
                                        kT[pb, i * 128:(i + 1) * 128],
                                        qT[t][pb, j * 512 + ec0:(j + 1) * 512],
                                        start=True, stop=True)
                                    es = ep.tile([128, 512], F32R, tag="es")
                                    nc.scalar.activation(
                                        es[:, c0:512], ss[:, c0:512], EXP, scale=SCALE)
                                    if i >= 4 * j:
                                        nc.vector.tensor_mul(
                                            es[:, c0:c0 + 128], es[:, c0:c0 + 128],
                                            trib[:])
                                    if c0 == 384:
                                        nc.vector.tensor_copy(es[:, 256:384], zblkb[:])
                                    nc.tensor.matmul(
                                        avs[sub][:, av0:512], vo[g][i][:],
                                        es[:, av0:512],
                                        start=(i == 0), stop=(i == 4 * j + 3))
                            for sub in range(2):
                                pb = slice(64 * sub, 64 * sub + 64)
                                nc.vector.tensor_copy(outT[t][pb, jcol], avs[sub][0:64, :])
                                dst = denomA if sub == 0 else denomB
                                nc.vector.tensor_copy(
                                    dst[32 * t:32 * t + 1, jcol], avs[sub][64:65, :])

            # ---------- Phase E: normalize + o_proj ----------
            with (
                tc.tile_pool(name="norm", bufs=2) as np_,
                tc.tile_pool(name="wop", bufs=1) as wop,
                tc.tile_pool(name="oout", bufs=3) as op,
                tc.tile_pool(name="bcps", bufs=2, space="PSUM") as bp_,
                tc.tile_pool(name="ops", bufs=4, space="PSUM") as opp,
                tc.tile_pool(name="rcp", bufs=1) as rcp,
            ):
                rcpf = rcp.tile([128, S], F32, tag="rcpf")
                rcprA = rcp.tile([128, S], F32R, tag="rcprA")
                rcprB = rcp.tile([128, S], F32R, tag="rcprB")
                for dt_, rr in ((denomA, rcprA), (denomB, rcprB)):
                    nc.vector.reciprocal(rcpf[:], dt_[:])
                    nc.vector.tensor_copy(rr[:], rcpf[:])
                for t in range(4):
                    tsl = slice(t * 128, (t + 1) * 128)
                    bcs = np_.tile([128, S], F32, tag="bcs")
                    for n in range(NCHUNK):
                        ncol = slice(n * 512, (n + 1) * 512)
                        bps = bp_.tile([128, 512], F32, tag="bps")
                        nc.tensor.matmul(
                            bps[:], selAb[:, tsl], rcprA[:, ncol],
                            start=True, stop=False)
                        nc.tensor.matmul(
                            bps[:], selBb[:, tsl], rcprB[:, ncol],
                            start=False, stop=True)
                        nc.vector.tensor_copy(bcs[:, ncol], bps[:])
                    nc.vector.tensor_mul(outT[t][:], outT[t][:], bcs[:])
                wot = [wop.tile([128, S], F32R, tag=f"wo{k}", name=f"wot{k}") for k in range(4)]
                for k in range(4):
                    nc.sync.dma_start(wot[k][:], wo[k * 128:(k + 1) * 128, :])
                for st in range(NST):
                    for dch in range(NCHUNK):
                        ops = opp.tile([128, 512], F32, tag="opps")
                        for kt in range(4):
                            nc.tensor.matmul(
                                ops[:], outT[kt][:, st * 128:(st + 1) * 128],
                                wot[kt][:, dch * 512:(dch + 1) * 512],
                                start=(kt == 0), stop=(kt == 3))
                        oo = op.tile([128, 512], F32, tag="oo")
                        nc.vector.tensor_copy(oo[:], ops[:])
                        nc.sync.dma_start(
                            opart[st * 128:(st + 1) * 128, dch * 512:(dch + 1) * 512],
                            oo[:])

    nc.compile()
    return nc


_PROGRAM = None


def _get_program():
    global _PROGRAM
    if _PROGRAM is None:
        _PROGRAM = _build_program()
    return _PROGRAM


def _make_in_maps(x, cos, sin, Wq, Wk, Wv, Wo):
    cosT = np.ascontiguousarray(cos.T.astype(np.float32))      # [64, S]
    sinT = np.ascontiguousarray(sin.T.astype(np.float32))
    cosT2 = np.tile(cosT, (2, 1))
    sinT2m = np.tile(np.concatenate([-sinT[:32], sinT[32:]], 0), (2, 1))
    tri = (np.arange(128)[None, :] >= np.arange(128)[:, None]).astype(np.float32)
    ident = np.tile(np.eye(64, dtype=np.float32), (2, 1))
    selA = np.zeros((128, 512), dtype=np.float32)
    selB = np.zeros((128, 512), dtype=np.float32)
    for t in range(4):
        selA[32 * t, 128 * t:128 * t + 64] = 1.0
        selB[32 * t, 128 * t + 64:128 * t + 128] = 1.0

    perm = [0, 4, 1, 5, 2, 6, 3, 7]
    in_maps = []
    for c in range(8):
        b, q = c // 4, c % 4
        idx = np.concatenate([np.arange(HD) + (8 * q + j) * HD for j in perm])
        in_maps.append({
            "xT": np.ascontiguousarray(x[b].T.astype(np.float32)),
            "wq": np.ascontiguousarray(Wq[:, idx].astype(np.float32)),
            "wk": np.ascontiguousarray(Wk[:, 2 * q * HD:(2 * q + 2) * HD].astype(np.float32)),
            "wv": np.ascontiguousarray(Wv[:, 2 * q * HD:(2 * q + 2) * HD].astype(np.float32)),
            "wo": np.ascontiguousarray(Wo[idx, :].astype(np.float32)),
            "cosT2": cosT2,
            "sinT2m": sinT2m,
            "tri": tri,
            "ident": ident,
            "selA": selA,
            "selB": selB,
            "onescol": np.ones((128, 1), dtype=np.float32),
            "zblk": np.zeros((128, 128), dtype=np.float32),
        })
    return in_maps


def _execute(in_maps, trace=False):
    nc = _get_program()
    return bass_utils.run_bass_kernel_spmd(
        nc, in_maps, core_ids=list(range(8)), trace=trace)


def kernel(x, cos, sin, Wq, Wk, Wv, Wo):
    in_maps = _make_in_maps(x, cos, sin, Wq, Wk, Wv, Wo)
    res = _execute(in_maps, trace=False)
    parts = [r["opart"] for r in res.results]
    out = np.empty((B, S, D), dtype=np.float32)
    for b in range(B):
        p = parts[4 * b:4 * b + 4]
        out[b] = (p[0] + p[1]) + (p[2] + p[3])
    return out



# revision 2
# speedup vs baseline: 1.0414x; 1.0414x over previous
"""GQA (RoPE + causal softmax) Trainium2 Bass kernel, 8-core SPMD. v4.

v4 = v3 + stall fixes from the v3 trace:
- reciprocal -> reciprocal_approx_fast (3.3us -> ~0.7us, was blocking the
  normalization matmuls for ~5us per chunk)
- normalization of chunk n-1 deferred into the middle of attention slice n
- attention-output and v evacuations moved to ScalarE (VectorE was the
  queue behind PSUM slot recycling at projection pass boundaries)
- v produced via 16 N=512 matmuls + 4 PE transposes per chunk instead of
  64 N=128 matmuls (~10us less PE)
- constant/wo DMAs moved behind the first x-chunk DMAs (faster cold start)
"""

import math
import numpy as np
import ml_dtypes

import concourse.bass as bass
import concourse.mybir as mybir
import concourse.tile as tile
from concourse import bacc, bass_utils

B, S, D = 2, 2048, 2048
H, KV, HD = 32, 8, 64
REP = H // KV
SCALE = 1.0 / 8.0

F32 = mybir.dt.float32
BF16 = mybir.dt.bfloat16
EXP = mybir.ActivationFunctionType.Exp

NCHUNK = S // 512
NKT = D // 128
BF = ml_dtypes.bfloat16


def _build_program():
    nc = bacc.Bacc()

    xT = nc.dram_tensor("xT", [D, S], BF16, kind="ExternalInput").ap()
    wq = nc.dram_tensor("wq", [D, 8 * HD], BF16, kind="ExternalInput").ap()
    wk = nc.dram_tensor("wk", [D, 2 * HD], BF16, kind="ExternalInput").ap()
    wv = nc.dram_tensor("wv", [D, 2 * HD], BF16, kind="ExternalInput").ap()
    wo = nc.dram_tensor("wo", [8 * HD, D], BF16, kind="ExternalInput").ap()
    cosT2 = nc.dram_tensor("cosT2", [128, S], BF16, kind="ExternalInput").ap()
    sinT2m = nc.dram_tensor("sinT2m", [128, S], BF16, kind="ExternalInput").ap()
    negtri = nc.dram_tensor("negtri", [128, 128], F32, kind="ExternalInput").ap()
    selA = nc.dram_tensor("selA", [128, 512], BF16, kind="ExternalInput").ap()
    selB = nc.dram_tensor("selB", [128, 512], BF16, kind="ExternalInput").ap()
    onescol = nc.dram_tensor("onescol", [128, 1], BF16, kind="ExternalInput").ap()
    ident128 = nc.dram_tensor("ident128", [128, 128], BF16, kind="ExternalInput").ap()
    opart = nc.dram_tensor("opart", [S, D], BF16, kind="ExternalOutput").ap()

    with tile.TileContext(nc) as tc:
        with (
            tc.tile_pool(name="persist", bufs=1) as pp,
            tc.tile_pool(name="consts", bufs=1) as cp,
            tc.tile_pool(name="wts", bufs=1) as wp,
            tc.tile_pool(name="xin", bufs=36) as xp,
            tc.tile_pool(name="rope", bufs=2) as rp,
            tc.tile_pool(name="vop", bufs=1) as vp,
            tc.tile_pool(name="esb", bufs=3) as ep,
            tc.tile_pool(name="normb", bufs=2) as np_,
            tc.tile_pool(name="wop", bufs=1) as wop,
            tc.tile_pool(name="oout", bufs=3) as op_,
            tc.tile_pool(name="accp", bufs=4, space="PSUM") as accp,
            tc.tile_pool(name="ssp", bufs=2, space="PSUM") as ssp,
        ):
            qT = [pp.tile([128, S], BF16, tag=f"qT{t}", name=f"qT{t}") for t in range(4)]
            kT = pp.tile([128, S], BF16, tag="kT")
            outT = [pp.tile([128, S], BF16, tag=f"outT{t}", name=f"outT{t}") for t in range(4)]
            denomA = pp.tile([128, S], F32, tag="denomA")
            denomB = pp.tile([128, S], F32, tag="denomB")
            rcpA = pp.tile([128, S], F32, tag="rcpA")
            rcpB = pp.tile([128, S], F32, tag="rcpB")
            rcpAb = pp.tile([128, S], BF16, tag="rcpAb")
            rcpBb = pp.tile([128, S], BF16, tag="rcpBb")
            trib = cp.tile([128, 128], F32, tag="trib")
            selAb = cp.tile([128, 512], BF16, tag="selAb")
            selBb = cp.tile([128, 512], BF16, tag="selBb")
            onesb = cp.tile([128, 1], BF16, tag="onesb")
            idb = cp.tile([128, 128], BF16, tag="idb")
            cosb = cp.tile([128, S], BF16, tag="cosb")
            sinb = cp.tile([128, S], BF16, tag="sinb")

            def const_dmas():
                nc.sync.dma_start(cosb[:], cosT2[:])
                nc.sync.dma_start(sinb[:], sinT2m[:])
                nc.sync.dma_start(trib[:], negtri[:])
                nc.sync.dma_start(selAb[:], selA[:])
                nc.sync.dma_start(selBb[:], selB[:])
                nc.sync.dma_start(onesb[:], onescol[:])
                nc.sync.dma_start(idb[:], ident128[:])
                nc.gpsimd.memset(denomA[:], 1.0)
                nc.gpsimd.memset(denomB[:], 1.0)

            wqk = [wp.tile([128, 8 * HD], BF16, tag=f"wq{k}", name=f"wqk{k}") for k in range(NKT)]
            wkk = [wp.tile([128, 2 * HD], BF16, tag=f"wk{k}", name=f"wkk{k}") for k in range(NKT)]
            wvk = [wp.tile([128, 2 * HD], BF16, tag=f"wv{k}", name=f"wvk{k}") for k in range(NKT)]
            wot = [wop.tile([128, S], BF16, tag=f"wo{k}", name=f"wot{k}")
                   for k in range(4)]
            vo = [[None] * (S // 128), [None] * (S // 128)]

            def rope_evac(dst_cols, acc, ncol):
                rotb = rp.tile([128, 512], BF16, tag="rot", name="rotb")
                nc.vector.tensor_mul(rotb[0:32, :], acc[32:64, :], sinb[0:32, ncol])
                nc.vector.tensor_mul(rotb[32:64, :], acc[0:32, :], sinb[32:64, ncol])
                nc.vector.tensor_mul(rotb[64:96, :], acc[96:128, :], sinb[64:96, ncol])
                nc.vector.tensor_mul(rotb[96:128, :], acc[64:96, :], sinb[96:128, ncol])
                nc.vector.tensor_mul(dst_cols, acc[:], cosb[:, ncol])
                nc.vector.tensor_add(dst_cols, dst_cols, rotb[:])

            def chunk_units(n):
                """Generator of emission units (each ~2 matmuls of PE work)
                for projection chunk n."""
                ncol = slice(n * 512, (n + 1) * 512)
                xk = [xp.tile([128, 512], BF16, tag="xk", name=f"xk{n}_{k}")
                      for k in range(NKT)]

                def dmas():
                    for k in range(NKT):
                        if n == 0:
                            r = slice(k * 128, (k + 1) * 128)
                            nc.sync.dma_start(wqk[k][:], wq[r, :])
                            nc.sync.dma_start(wkk[k][:], wk[r, :])
                            nc.sync.dma_start(wvk[k][:], wv[r, :])
                        nc.sync.dma_start(xk[k][:], xT[k * 128:(k + 1) * 128, ncol])
                    if n == 0:
                        const_dmas()
                    if n == 1:
                        for k in range(4):
                            nc.sync.dma_start(wot[k][:], wo[k * 128:(k + 1) * 128, :])
                yield dmas

                for p in range(2):
                    accs = [accp.tile([128, 512], F32, tag="acc",
                                      name=f"acc{n}_{p}_{m}") for m in range(2)]

                    def qmm(k, p=p, accs=accs):
                        st, sp_ = k == 0, k == NKT - 1
                        for m in range(2):
                            t = 2 * p + m
                            nc.tensor.matmul(
                                accs[m][:], wqk[k][:, t * 128:(t + 1) * 128],
                                xk[k][:], start=st, stop=sp_)
                    for k in range(NKT):
                        yield (lambda k=k, f=qmm: f(k))

                    def qevac(p=p, accs=accs):
                        for m in range(2):
                            rope_evac(qT[2 * p + m][:, ncol], accs[m], ncol)
                    yield qevac

                kacc = accp.tile([128, 512], F32, tag="acc", name=f"kacc{n}")

                def kmm(k2):
                    nc.tensor.matmul(kacc[:], wkk[k2][:], xk[k2][:],
                                     start=(k2 == 0), stop=(k2 == NKT - 1))
                for k in range(0, NKT, 2):
                    yield (lambda k=k: (kmm(k), kmm(k + 1)))
                yield lambda: rope_evac(kT[:, ncol], kacc, ncol)

                # v^T chunk, then PE-transpose each 128-block to [kpos, hd]
                vacc = accp.tile([128, 512], F32, tag="acc", name=f"vacc{n}")

                def vmm(k2):
                    nc.tensor.matmul(vacc[:], wvk[k2][:], xk[k2][:],
                                     start=(k2 == 0), stop=(k2 == NKT - 1))
                for k in range(0, NKT, 2):
                    yield (lambda k=k: (vmm(k), vmm(k + 1)))

                vTs = rp.tile([128, 512], BF16, tag="vTs", name=f"vTs{n}")
                yield lambda: nc.vector.tensor_copy(vTs[:], vacc[:])

                for iq in range(4):
                    i = 4 * n + iq

                    def vtrans(iq=iq, i=i):
                        tps = accp.tile([128, 128], BF16, tag="acc",
                                        name=f"tps{n}_{iq}")
                        nc.tensor.transpose(
                            tps[:], vTs[:, iq * 128:(iq + 1) * 128], idb[:])
                        for g in range(2):
                            vot = vp.tile([128, HD + 1], BF16, tag=f"vo{g}_{i}",
                                          name=f"vo{g}_{i}")
                            nc.scalar.copy(
                                vot[:, 0:HD], tps[:, g * HD:(g + 1) * HD])
                            nc.scalar.copy(vot[:, HD:HD + 1], onesb[:])
                            vo[g][i] = vot
                    yield vtrans

            def oproj_unit(st, dch):
                ops = accp.tile([128, 512], F32, tag="acc",
                                name=f"ops{st}_{dch}")
                for kt in range(4):
                    nc.tensor.matmul(
                        ops[:], outT[kt][:, st * 128:(st + 1) * 128],
                        wot[kt][:, dch * 512:(dch + 1) * 512],
                        start=(kt == 0), stop=(kt == 3))
                oo = op_.tile([128, 512], BF16, tag="oo", name=f"oo{st}_{dch}")
                nc.vector.tensor_copy(oo[:], ops[:])
                nc.sync.dma_start(
                    opart[st * 128:(st + 1) * 128, dch * 512:(dch + 1) * 512],
                    oo[:])

            def emit_norm(m):
                mcol = slice(m * 512, (m + 1) * 512)
                for t in range(4):
                    tsl = slice(t * 128, (t + 1) * 128)
                    bps = ssp.tile([128, 2, 512], F32, tag="scps",
                                   name=f"bps{m}_{t}")
                    nc.tensor.matmul(bps[:, 0, :], selAb[:, tsl],
                                     rcpAb[:, mcol], start=True, stop=False)
                    nc.tensor.matmul(bps[:, 0, :], selBb[:, tsl],
                                     rcpBb[:, mcol], start=False, stop=True)
                    bcs = np_.tile([128, 512], F32, tag="bcs",
                                   name=f"bcs{m}_{t}")
                    nc.scalar.copy(bcs[:], bps[:, 0, :])
                    nc.vector.tensor_mul(outT[t][:, mcol], outT[t][:, mcol],
                                         bcs[:])

            def emit_av(avs, pend, j):
                pi, pc0, pes = pend
                for g in range(2):
                    nc.tensor.matmul(
                        avs[g][0:HD + 1, pc0:512], vo[g][pi][:],
                        pes[:, g, pc0:512],
                        start=(pi == 0), stop=(pi == 4 * j + 3))

            # ---------------- main schedule ----------------
            for u in chunk_units(0):
                u()

            for n in range(NCHUNK):
                if n < NCHUNK - 1:
                    filler = list(chunk_units(n + 1))
                else:
                    filler = [
                        (lambda st=st, dch=dch: oproj_unit(st, dch))
                        for st in range(12) for dch in range(4)
                    ]

                j = n
                nblocks = 16 * (n + 1)
                bi = 0
                for t in range(4):
                    avs = [accp.tile([128, 512], F32, tag="acc",
                                     name=f"av{t}_{j}_{s}") for s in range(2)]
                    pend = None
                    for i in range(4 * j + 4):
                        c0 = max(0, 128 * (i - 4 * j))
                        ss = ssp.tile([128, 2, 512], F32, tag="scps",
                                      name=f"ss{t}_{j}_{i}")
                        for g in range(2):
                            pb = slice(64 * g, 64 * g + 64)
                            nc.tensor.matmul(
                                ss[:, g, c0:512],
                                kT[pb, i * 128:(i + 1) * 128],
                                qT[t][pb, j * 512 + c0:(j + 1) * 512],
                                start=True, stop=True)
                        if i >= 4 * j:
                            for g in range(2):
                                nc.vector.tensor_add(
                                    ss[:, g, c0:c0 + 128],
                                    ss[:, g, c0:c0 + 128], trib[:])
                        es = ep.tile([128, 2, 512], BF16, tag="es",
                                     name=f"es{t}_{j}_{i}")
                        nc.scalar.activation(
                            es[:, :, c0:512], ss[:, :, c0:512], EXP,
                            scale=SCALE)
                        if pend is not None:
                            emit_av(avs, pend, j)
                        pend = (i, c0, es)
                        # PE filler: spread next chunk / o_proj between blocks
                        bi += 1
                        nu = math.ceil(len(filler) * bi / nblocks) - (
                            math.ceil(len(filler) * (bi - 1) / nblocks))
                        for _ in range(nu):
                            filler.pop(0)()
                    emit_av(avs, pend, j)
                    jcol = slice(j * 512, (j + 1) * 512)
                    # g=0 same-base copy on ScalarE; g=1 is cross-partition
                    # (0->64) which is only proven on VectorE
                    nc.scalar.copy(outT[t][0:64, jcol], avs[0][0:HD, :])
                    nc.vector.tensor_copy(outT[t][64:128, jcol], avs[1][0:HD, :])
                    for g in range(2):
                        dst = denomA if g == 0 else denomB
                        nc.vector.tensor_copy(
                            dst[32 * t:32 * t + 1, jcol], avs[g][HD:HD + 1, :])
                    if t == 0 and n > 0:
                        emit_norm(n - 1)

                for u in filler:
                    u()
                ncol = slice(n * 512, (n + 1) * 512)
                nc.vector.reciprocal_approx_fast(rcpA[:, ncol], denomA[:, ncol])
                nc.vector.reciprocal_approx_fast(rcpB[:, ncol], denomB[:, ncol])
                nc.vector.tensor_copy(rcpAb[:, ncol], rcpA[:, ncol])
                nc.vector.tensor_copy(rcpBb[:, ncol], rcpB[:, ncol])

            emit_norm(NCHUNK - 1)
            for st in range(12, 16):
                for dch in range(4):
                    oproj_unit(st, dch)

    nc.compile()
    return nc


_PROGRAM = None


def _get_program():
    global _PROGRAM
    if _PROGRAM is None:
        _PROGRAM = _build_program()
    return _PROGRAM


def _make_in_maps(x, cos, sin, Wq, Wk, Wv, Wo):
    cosT = np.ascontiguousarray(cos.T.astype(np.float32))
    sinT = np.ascontiguousarray(sin.T.astype(np.float32))
    cosT2 = np.tile(cosT, (2, 1)).astype(BF)
    sinT2m = np.tile(np.concatenate([-sinT[:32], sinT[32:]], 0), (2, 1)).astype(BF)
    valid = np.arange(128)[None, :] >= np.arange(128)[:, None]
    negtri = np.where(valid, 0.0, -1e30).astype(np.float32)
    selA = np.zeros((128, 512), dtype=np.float32)
    selB = np.zeros((128, 512), dtype=np.float32)
    for t in range(4):
        selA[32 * t, 128 * t:128 * t + 64] = 1.0
        selB[32 * t, 128 * t + 64:128 * t + 128] = 1.0
    selA, selB = selA.astype(BF), selB.astype(BF)
    ones = np.ones((128, 1), dtype=np.float32).astype(BF)
    ident128 = np.eye(128, dtype=np.float32).astype(BF)

    perm = [0, 4, 1, 5, 2, 6, 3, 7]
    xTb = [np.ascontiguousarray(x[b].T).astype(BF) for b in range(B)]
    in_maps = []
    for c in range(8):
        b, q = c // 4, c % 4
        idx = np.concatenate([np.arange(HD) + (8 * q + j) * HD for j in perm])
        in_maps.append({
            "xT": xTb[b],
            "wq": np.ascontiguousarray(Wq[:, idx]).astype(BF),
            "wk": np.ascontiguousarray(Wk[:, 2 * q * HD:(2 * q + 2) * HD]).astype(BF),
            "wv": np.ascontiguousarray(Wv[:, 2 * q * HD:(2 * q + 2) * HD]).astype(BF),
            "wo": np.ascontiguousarray(Wo[idx, :]).astype(BF),
            "cosT2": cosT2,
            "sinT2m": sinT2m,
            "negtri": negtri,
            "selA": selA,
            "selB": selB,
            "onescol": ones,
            "ident128": ident128,
        })
    return in_maps


def _execute(in_maps, trace=False):
    nc = _get_program()
    return bass_utils.run_bass_kernel_spmd(
        nc, in_maps, core_ids=list(range(8)), trace=trace)


def kernel(x, cos, sin, Wq, Wk, Wv, Wo):
    in_maps = _make_in_maps(x, cos, sin, Wq, Wk, Wv, Wo)
    res = _execute(in_maps, trace=False)
    parts = [r["opart"].astype(np.float32) for r in res.results]
    out = np.empty((B, S, D), dtype=np.float32)
    for b in range(B):
        p = parts[4 * b:4 * b + 4]
        out[b] = (p[0] + p[1]) + (p[2] + p[3])
    return out


# revision 3
# speedup vs baseline: 1.1210x; 1.0764x over previous
"""GQA (RoPE + causal softmax) Trainium2 Bass kernel, 8-core SPMD. v6.

v4 = v3 + stall fixes from the v3 trace:
- reciprocal -> reciprocal_approx_fast (3.3us -> ~0.7us, was blocking the
  normalization matmuls for ~5us per chunk)
- normalization of chunk n-1 deferred into the middle of attention slice n
- attention-output and v evacuations moved to ScalarE (VectorE was the
  queue behind PSUM slot recycling at projection pass boundaries)
- v produced via 16 N=512 matmuls + 4 PE transposes per chunk instead of
  64 N=128 matmuls (~10us less PE)
- constant/wo DMAs moved behind the first x-chunk DMAs (faster cold start)
"""

import math
import numpy as np
import ml_dtypes

import concourse.bass as bass
import concourse.mybir as mybir
import concourse.tile as tile
from concourse import bacc, bass_utils


B, S, D = 2, 2048, 2048
H, KV, HD = 32, 8, 64
REP = H // KV
SCALE = 1.0 / 8.0

F32 = mybir.dt.float32
BF16 = mybir.dt.bfloat16
EXP = mybir.ActivationFunctionType.Exp

NCHUNK = S // 512
NKT = D // 128
BF = ml_dtypes.bfloat16


def _build_program():
    nc = bacc.Bacc()

    xT = nc.dram_tensor("xT", [D, S], BF16, kind="ExternalInput").ap()
    wq = nc.dram_tensor("wq", [D, 8 * HD], BF16, kind="ExternalInput").ap()
    wk = nc.dram_tensor("wk", [D, 2 * HD], BF16, kind="ExternalInput").ap()
    wv = nc.dram_tensor("wv", [D, 2 * HD], BF16, kind="ExternalInput").ap()
    wo = nc.dram_tensor("wo", [8 * HD, D], BF16, kind="ExternalInput").ap()
    cosT2 = nc.dram_tensor("cosT2", [128, S], BF16, kind="ExternalInput").ap()
    sinT2m = nc.dram_tensor("sinT2m", [128, S], BF16, kind="ExternalInput").ap()
    negtri = nc.dram_tensor("negtri", [128, 128], BF16, kind="ExternalInput").ap()
    selA = nc.dram_tensor("selA", [128, 512], BF16, kind="ExternalInput").ap()
    selB = nc.dram_tensor("selB", [128, 512], BF16, kind="ExternalInput").ap()
    onescol = nc.dram_tensor("onescol", [128, 1], BF16, kind="ExternalInput").ap()
    ident128 = nc.dram_tensor("ident128", [128, 128], BF16, kind="ExternalInput").ap()
    opart = nc.dram_tensor("opart", [S, D], BF16, kind="ExternalOutput").ap()

    with tile.TileContext(nc) as tc:
        with (
            tc.tile_pool(name="persist", bufs=1) as pp,
            tc.tile_pool(name="consts", bufs=1) as cp,
            tc.tile_pool(name="wts", bufs=1) as wp,
            tc.tile_pool(name="xin", bufs=36) as xp,
            tc.tile_pool(name="rope", bufs=2) as rp,
            tc.tile_pool(name="vop", bufs=1) as vp,
            tc.tile_pool(name="esb", bufs=3) as ep,
            tc.tile_pool(name="normb", bufs=2) as np_,
            tc.tile_pool(name="wop", bufs=1) as wop,
            tc.tile_pool(name="oout", bufs=3) as op_,
            tc.tile_pool(name="accp", bufs=4, space="PSUM") as accp,
            tc.tile_pool(name="ssp", bufs=2, space="PSUM") as ssp,
        ):
            qT = [pp.tile([128, S], BF16, tag=f"qT{t}", name=f"qT{t}") for t in range(4)]
            kT = pp.tile([128, S], BF16, tag="kT")
            outT = [pp.tile([128, S], BF16, tag=f"outT{t}", name=f"outT{t}") for t in range(4)]
            denomA = pp.tile([128, S], F32, tag="denomA")
            denomB = pp.tile([128, S], F32, tag="denomB")
            rcpA = pp.tile([128, S], F32, tag="rcpA")
            rcpB = pp.tile([128, S], F32, tag="rcpB")
            rcpAb = pp.tile([128, S], BF16, tag="rcpAb")
            rcpBb = pp.tile([128, S], BF16, tag="rcpBb")
            trib = cp.tile([128, 128], BF16, tag="trib5")
            selAb = cp.tile([128, 512], BF16, tag="selAb")
            selBb = cp.tile([128, 512], BF16, tag="selBb")
            onesb = cp.tile([128, 1], BF16, tag="onesb")
            idb = cp.tile([128, 128], BF16, tag="idb")
            cosb = cp.tile([128, S], BF16, tag="cosb")
            sinb = cp.tile([128, S], BF16, tag="sinb")

            def const_dmas():
                nc.sync.dma_start(cosb[:], cosT2[:])
                nc.sync.dma_start(sinb[:], sinT2m[:])
                nc.sync.dma_start(trib[:], negtri[:])
                nc.sync.dma_start(selAb[:], selA[:])
                nc.sync.dma_start(selBb[:], selB[:])
                nc.sync.dma_start(onesb[:], onescol[:])
                nc.sync.dma_start(idb[:], ident128[:])
                nc.gpsimd.memset(denomA[:], 1.0)
                nc.gpsimd.memset(denomB[:], 1.0)

            wqk = [wp.tile([128, 8 * HD], BF16, tag=f"wq{k}", name=f"wqk{k}") for k in range(NKT)]
            wkk = [wp.tile([128, 2 * HD], BF16, tag=f"wk{k}", name=f"wkk{k}") for k in range(NKT)]
            wvk = [wp.tile([128, 2 * HD], BF16, tag=f"wv{k}", name=f"wvk{k}") for k in range(NKT)]
            wot = [wop.tile([128, S], BF16, tag=f"wo{k}", name=f"wot{k}")
                   for k in range(4)]
            vo = [[None] * (S // 128), [None] * (S // 128)]

            def rope_evac(dst_cols, acc, ncol):
                rotb = rp.tile([128, 512], BF16, tag="rot", name="rotb")
                nc.vector.tensor_mul(rotb[0:32, :], acc[32:64, :], sinb[0:32, ncol])
                nc.vector.tensor_mul(rotb[32:64, :], acc[0:32, :], sinb[32:64, ncol])
                nc.vector.tensor_mul(rotb[64:96, :], acc[96:128, :], sinb[64:96, ncol])
                nc.vector.tensor_mul(rotb[96:128, :], acc[64:96, :], sinb[96:128, ncol])
                nc.vector.tensor_mul(dst_cols, acc[:], cosb[:, ncol])
                nc.vector.tensor_add(dst_cols, dst_cols, rotb[:])

            def chunk_units(n):
                """Generator of emission units (each ~2 matmuls of PE work)
                for projection chunk n."""
                ncol = slice(n * 512, (n + 1) * 512)
                xk = [xp.tile([128, 512], BF16, tag="xk", name=f"xk{n}_{k}")
                      for k in range(NKT)]

                def dmas():
                    for k in range(NKT):
                        if n == 0:
                            r = slice(k * 128, (k + 1) * 128)
                            nc.sync.dma_start(wqk[k][:], wq[r, :])
                            nc.sync.dma_start(wkk[k][:], wk[r, :])
                            nc.sync.dma_start(wvk[k][:], wv[r, :])
                        nc.sync.dma_start(xk[k][:], xT[k * 128:(k + 1) * 128, ncol])
                    if n == 0:
                        const_dmas()
                    if n == 1:
                        for k in range(4):
                            nc.sync.dma_start(wot[k][:], wo[k * 128:(k + 1) * 128, :])
                yield dmas

                for p in range(2):
                    accs = [accp.tile([128, 512], F32, tag="acc",
                                      name=f"acc{n}_{p}_{m}") for m in range(2)]

                    def qmm(k, p=p, accs=accs):
                        st, sp_ = k == 0, k == NKT - 1
                        for m in range(2):
                            t = 2 * p + m
                            nc.tensor.matmul(
                                accs[m][:], wqk[k][:, t * 128:(t + 1) * 128],
                                xk[k][:], start=st, stop=sp_)
                    for k in range(NKT):
                        yield (lambda k=k, f=qmm: f(k))

                    def qevac(p=p, accs=accs):
                        for m in range(2):
                            rope_evac(qT[2 * p + m][:, ncol], accs[m], ncol)
                    yield qevac

                kacc = accp.tile([128, 512], F32, tag="acc", name=f"kacc{n}")

                def kmm(k2):
                    nc.tensor.matmul(kacc[:], wkk[k2][:], xk[k2][:],
                                     start=(k2 == 0), stop=(k2 == NKT - 1))
                for k in range(0, NKT, 2):
                    yield (lambda k=k: (kmm(k), kmm(k + 1)))
                yield lambda: rope_evac(kT[:, ncol], kacc, ncol)

                # v^T chunk, then PE-transpose each 128-block to [kpos, hd]
                vacc = accp.tile([128, 512], F32, tag="acc", name=f"vacc{n}")

                def vmm(k2):
                    nc.tensor.matmul(vacc[:], wvk[k2][:], xk[k2][:],
                                     start=(k2 == 0), stop=(k2 == NKT - 1))
                for k in range(0, NKT, 2):
                    yield (lambda k=k: (vmm(k), vmm(k + 1)))

                vTs = rp.tile([128, 512], BF16, tag="vTs", name=f"vTs{n}")
                yield lambda: nc.vector.tensor_copy(vTs[:], vacc[:])

                for iq in range(4):
                    i = 4 * n + iq

                    def vtrans(iq=iq, i=i):
                        tps = accp.tile([128, 128], BF16, tag="acc",
                                        name=f"tps{n}_{iq}")
                        nc.tensor.transpose(
                            tps[:], vTs[:, iq * 128:(iq + 1) * 128], idb[:])
                        for g in range(2):
                            vot = vp.tile([128, HD + 1], BF16, tag=f"vo{g}_{i}",
                                          name=f"vo{g}_{i}")
                            nc.scalar.copy(
                                vot[:, 0:HD], tps[:, g * HD:(g + 1) * HD])
                            nc.scalar.copy(vot[:, HD:HD + 1], onesb[:])
                            vo[g][i] = vot
                    yield vtrans

            def oproj_unit(st, dch):
                ops = accp.tile([128, 512], F32, tag="acc",
                                name=f"ops{st}_{dch}")
                for kt in range(4):
                    nc.tensor.matmul(
                        ops[:], outT[kt][:, st * 128:(st + 1) * 128],
                        wot[kt][:, dch * 512:(dch + 1) * 512],
                        start=(kt == 0), stop=(kt == 3))
                oo = op_.tile([128, 512], BF16, tag="oo", name=f"oo{st}_{dch}")
                nc.vector.tensor_copy(oo[:], ops[:])
                nc.sync.dma_start(
                    opart[st * 128:(st + 1) * 128, dch * 512:(dch + 1) * 512],
                    oo[:])

            def emit_norm(m):
                mcol = slice(m * 512, (m + 1) * 512)
                for t in range(4):
                    tsl = slice(t * 128, (t + 1) * 128)
                    bps = ssp.tile([128, 2, 512], F32, tag="scps",
                                   name=f"bps{m}_{t}")
                    nc.tensor.matmul(bps[:, 0, :], selAb[:, tsl],
                                     rcpAb[:, mcol], start=True, stop=False)
                    nc.tensor.matmul(bps[:, 0, :], selBb[:, tsl],
                                     rcpBb[:, mcol], start=False, stop=True)
                    bcs = np_.tile([128, 512], F32, tag="bcs",
                                   name=f"bcs{m}_{t}")
                    nc.scalar.copy(bcs[:], bps[:, 0, :])
                    nc.vector.tensor_mul(outT[t][:, mcol], outT[t][:, mcol],
                                         bcs[:])

            def emit_av(avs, pend, j):
                pi, pc0, pes = pend
                for g in range(2):
                    nc.tensor.matmul(
                        avs[g][0:HD + 1, pc0:512], vo[g][pi][:],
                        pes[:, g, pc0:512],
                        start=(pi == 0), stop=(pi == 4 * j + 3))

            # ---------------- main schedule ----------------
            for u in chunk_units(0):
                u()

            for n in range(NCHUNK):
                if n < NCHUNK - 1:
                    filler = list(chunk_units(n + 1))
                else:
                    filler = [
                        (lambda st=st, dch=dch: oproj_unit(st, dch))
                        for st in range(12) for dch in range(4)
                    ]

                j = n
                nblocks = 16 * (n + 1)
                bi = 0
                for t in range(4):
                    avs = [accp.tile([128, 512], F32, tag="acc",
                                     name=f"av{t}_{j}_{s}") for s in range(2)]
                    pend = None
                    for i in range(4 * j + 4):
                        c0 = max(0, 128 * (i - 4 * j))
                        ss = ssp.tile([128, 2, 512], F32, tag="scps",
                                      name=f"ss{t}_{j}_{i}")
                        for g in range(2):
                            pb = slice(64 * g, 64 * g + 64)
                            nc.tensor.matmul(
                                ss[:, g, c0:512],
                                kT[pb, i * 128:(i + 1) * 128],
                                qT[t][pb, j * 512 + c0:(j + 1) * 512],
                                start=True, stop=True)
                        es = ep.tile([128, 2, 512], BF16, tag="es",
                                     name=f"es{t}_{j}_{i}")
                        nc.scalar.activation(
                            es[:, :, c0:512], ss[:, :, c0:512], EXP,
                            scale=SCALE)
                        if i >= 4 * j:
                            # causal mask: zero the upper triangle post-exp
                            for g in range(2):
                                nc.vector.tensor_mul(
                                    es[:, g, c0:c0 + 128],
                                    es[:, g, c0:c0 + 128], trib[:])
                        if pend is not None:
                            emit_av(avs, pend, j)
                        pend = (i, c0, es)
                        # PE filler: spread next chunk / o_proj between blocks
                        bi += 1
                        nu = math.ceil(len(filler) * bi / nblocks) - (
                            math.ceil(len(filler) * (bi - 1) / nblocks))
                        for _ in range(nu):
                            filler.pop(0)()
                    emit_av(avs, pend, j)
                    jcol = slice(j * 512, (j + 1) * 512)
                    # g=0 same-base copy on ScalarE; g=1 is cross-partition
                    # (0->64) which is only proven on VectorE
                    nc.scalar.copy(outT[t][0:64, jcol], avs[0][0:HD, :])
                    nc.vector.tensor_copy(outT[t][64:128, jcol], avs[1][0:HD, :])
                    for g in range(2):
                        dst = denomA if g == 0 else denomB
                        nc.vector.tensor_copy(
                            dst[32 * t:32 * t + 1, jcol], avs[g][HD:HD + 1, :])
                    if t == 0 and n > 0:
                        emit_norm(n - 1)

                for u in filler:
                    u()
                ncol = slice(n * 512, (n + 1) * 512)
                nc.vector.reciprocal_approx_fast(rcpA[:, ncol], denomA[:, ncol])
                nc.vector.reciprocal_approx_fast(rcpB[:, ncol], denomB[:, ncol])
                nc.vector.tensor_copy(rcpAb[:, ncol], rcpA[:, ncol])
                nc.vector.tensor_copy(rcpBb[:, ncol], rcpB[:, ncol])

            emit_norm(NCHUNK - 1)
            for st in range(12, 16):
                for dch in range(4):
                    oproj_unit(st, dch)

    nc.compile()
    return nc


_PROGRAM = None


def _get_program():
    global _PROGRAM
    if _PROGRAM is None:
        _PROGRAM = _build_program()
    return _PROGRAM


def _make_in_maps(x, cos, sin, Wq, Wk, Wv, Wo):
    cosT = np.ascontiguousarray(cos.T.astype(np.float32))
    sinT = np.ascontiguousarray(sin.T.astype(np.float32))
    cosT2 = np.tile(cosT, (2, 1)).astype(BF)
    sinT2m = np.tile(np.concatenate([-sinT[:32], sinT[32:]], 0), (2, 1)).astype(BF)
    valid = np.arange(128)[None, :] >= np.arange(128)[:, None]
    negtri = valid.astype(np.float32).astype(BF)
    selA = np.zeros((128, 512), dtype=np.float32)
    selB = np.zeros((128, 512), dtype=np.float32)
    for t in range(4):
        selA[32 * t, 128 * t:128 * t + 64] = 1.0
        selB[32 * t, 128 * t + 64:128 * t + 128] = 1.0
    selA, selB = selA.astype(BF), selB.astype(BF)
    ones = np.ones((128, 1), dtype=np.float32).astype(BF)
    ident128 = np.eye(128, dtype=np.float32).astype(BF)

    perm = [0, 4, 1, 5, 2, 6, 3, 7]
    xTb = [np.ascontiguousarray(x[b].T).astype(BF) for b in range(B)]
    in_maps = []
    for c in range(8):
        b, q = c // 4, c % 4
        idx = np.concatenate([np.arange(HD) + (8 * q + j) * HD for j in perm])
        in_maps.append({
            "xT": xTb[b],
            "wq": np.ascontiguousarray(Wq[:, idx]).astype(BF),
            "wk": np.ascontiguousarray(Wk[:, 2 * q * HD:(2 * q + 2) * HD]).astype(BF),
            "wv": np.ascontiguousarray(Wv[:, 2 * q * HD:(2 * q + 2) * HD]).astype(BF),
            "wo": np.ascontiguousarray(Wo[idx, :]).astype(BF),
            "cosT2": cosT2,
            "sinT2m": sinT2m,
            "negtri": negtri,
            "selA": selA,
            "selB": selB,
            "onescol": ones,
            "ident128": ident128,
        })
    return in_maps


def _execute(in_maps, trace=False):
    nc = _get_program()
    return bass_utils.run_bass_kernel_spmd(
        nc, in_maps, core_ids=list(range(8)), trace=trace)


def kernel(x, cos, sin, Wq, Wk, Wv, Wo):
    in_maps = _make_in_maps(x, cos, sin, Wq, Wk, Wv, Wo)
    res = _execute(in_maps, trace=False)
    parts = [r["opart"].astype(np.float32) for r in res.results]
    out = np.empty((B, S, D), dtype=np.float32)
    for b in range(B):
        p = parts[4 * b:4 * b + 4]
        out[b] = (p[0] + p[1]) + (p[2] + p[3])
    return out
